# revision 9
# baseline (speedup 1.0000x reference)
"""Trainium2 Bass kernel for nn_BlockEnd_53266184405691.

Computes, for b in [0, 4096):
    y[b] = relu(residual[b] @ w + node[b]) row-masked so rows a >= M_b are 0
with B=4096, A=RF=F=128, fp32.

Strategy: pure data parallel over the batch dim across 8 NeuronCores
(512 batches/core). On each core, each batch is one 128x128 tile:
    psum = residual_b^T.T @ w        (PE, fp32)
    z    = psum + node_b             (DVE)
    out  = relu(z * mask_col)        (ACT; mask in {0,1} per partition, so
                                      relu(m*z) == m*relu(z))
Inputs are host-rearranged to chunk-major [chunk, 128-partition, free]
layouts so every DMA is a fully contiguous ~2MB transfer (DMA-efficiency:
>=1MiB per dma_start, 8KB contiguous per partition).
"""

import numpy as np

B, A, RF, F = 4096, 128, 128, 128
NCORES = 8
BSHARD = B // NCORES            # 512 batches per core
JB = 16                         # batches per chunk
NCHUNK = BSHARD // JB           # 32 chunks
CW = JB * F                     # 2048 free-dim elements per chunk tile

_nc_cache = {}


def _build_nc(nchunk=NCHUNK, repeat=1):
    import concourse.bacc as bacc
    import concourse.mybir as mybir
    import concourse.tile as tile

    dt = mybir.dt.float32
    nb = nchunk * JB  # batches per core this build

    # Bacc (not raw Bass): its compile() runs move_matmul_waits_to_ldweights
    # + generate_event_semaphores, which legalize multi-sem waits down to the
    # 1-wait-per-instruction TRN2 codegen limit.
    nc = bacc.Bacc("TRN2", target_bir_lowering=False, debug=False,
                   num_devices=NCORES)
    nodec = nc.dram_tensor("nodec", [nchunk, A, CW], dt, kind="ExternalInput")
    residc = nc.dram_tensor("residc", [nchunk, RF, CW], dt, kind="ExternalInput")
    w_d = nc.dram_tensor("w", [RF, F], dt, kind="ExternalInput")
    maskt = nc.dram_tensor("maskt", [A, nb], dt, kind="ExternalInput")
    outc = nc.dram_tensor("outc", [nchunk, A, CW], dt, kind="ExternalOutput")

    with tile.TileContext(nc) as tc:
        with (
            tc.tile_pool(name="const", bufs=1) as constp,
            tc.tile_pool(name="node", bufs=3) as nodep,
            tc.tile_pool(name="resid", bufs=3) as residp,
            tc.tile_pool(name="out", bufs=3) as outp,
            tc.tile_pool(name="z", bufs=6) as zp,
            tc.tile_pool(name="psum", bufs=6, space="PSUM") as psump,
        ):
            w_sb = constp.tile([RF, F], dt)
            nc.sync.dma_start(w_sb[:], w_d[:])
            mask_sb = constp.tile([A, nb], dt)
            nc.sync.dma_start(mask_sb[:], maskt[:])

            def body():
                for c in range(nchunk):
                    n_t = nodep.tile([A, CW], dt)
                    nc.sync.dma_start(n_t[:], nodec[c])
                    r_t = residp.tile([RF, CW], dt)
                    nc.sync.dma_start(r_t[:], residc[c])
                    o_t = outp.tile([A, CW], dt)
                    for g in range(JB // 4):
                        ps = psump.tile([A, 4 * F], dt)  # one PSUM bank: 4 batches
                        for u in range(4):
                            j = g * 4 + u
                            nc.tensor.matmul(
                                ps[:, u * F:(u + 1) * F],
                                r_t[:, j * A:(j + 1) * A],
                                w_sb[:],
                                start=True, stop=True,
                            )
                        z = zp.tile([A, 4 * F], dt)
                        nc.vector.tensor_add(
                            z[:], ps[:], n_t[:, g * 4 * F:(g + 1) * 4 * F])
                        for u in range(4):
                            j = g * 4 + u
                            gb = c * JB + j
                            nc.scalar.activation(
                                o_t[:, j * F:(j + 1) * F],
                                z[:, u * F:(u + 1) * F],
                                mybir.ActivationFunctionType.Relu,
                                scale=mask_sb[:, gb:gb + 1],
                            )
                    nc.scalar.dma_start(outc[c], o_t[:])

            if repeat == 1:
                body()
            else:
                # On-device timing loop: output is overwritten identically
                # each iteration, so the kernel stays correct.
                with tc.For_i(0, repeat, 1):
                    body()
    nc.finalize()
    return nc


def _get_nc(nchunk=NCHUNK, repeat=1):
    key = (nchunk, repeat)
    if key not in _nc_cache:
        _nc_cache[key] = _build_nc(nchunk, repeat)
    return _nc_cache[key]


def _prep_inputs(node_features, residual_features, w, mol_slice, nchunk=NCHUNK):
    """Shard + rearrange to chunk-major per-core layouts."""
    node_features = np.ascontiguousarray(node_features, dtype=np.float32)
    residual_features = np.ascontiguousarray(residual_features, dtype=np.float32)
    w = np.ascontiguousarray(w, dtype=np.float32)
    M = np.asarray(mol_slice)[:, 0].astype(np.int64)

    nb = nchunk * JB
    ncores = node_features.shape[0] // nb

    # nodec[i, c, a, j*F+f] = node[i*nb + c*JB + j, a, f]
    nodec = np.ascontiguousarray(
        node_features.reshape(ncores, nchunk, JB, A, F)
        .transpose(0, 1, 3, 2, 4)
        .reshape(ncores, nchunk, A, CW)
    )
    # residc[i, c, r, j*A+a] = residual[i*nb + c*JB + j, a, r]
    residc = np.ascontiguousarray(
        residual_features.reshape(ncores, nchunk, JB, A, RF)
        .transpose(0, 1, 4, 2, 3)
        .reshape(ncores, nchunk, RF, CW)
    )
    # maskt[i, a, jj] = 1.0 if a < M[i*nb + jj]
    maskt = np.ascontiguousarray(
        (np.arange(A, dtype=np.int64)[None, :, None]
         < M.reshape(ncores, nb)[:, None, :]).astype(np.float32)
    )
    in_maps = [
        {"nodec": nodec[i], "residc": residc[i], "w": w, "maskt": maskt[i]}
        for i in range(ncores)
    ]
    return in_maps


def _postprocess(results, nchunk=NCHUNK):
    outs = []
    for r in results:
        o = np.asarray(r["outc"], dtype=np.float32)
        outs.append(
            o.reshape(nchunk, A, JB, F).transpose(0, 2, 1, 3).reshape(nchunk * JB, A, F)
        )
    return np.concatenate(outs, axis=0)


def run(node_features, residual_features, w, mol_slice, nchunk=NCHUNK, repeat=1,
        **spmd_kwargs):
    from concourse.bass_utils import run_bass_kernel_spmd

    nc = _get_nc(nchunk, repeat)
    in_maps = _prep_inputs(node_features, residual_features, w, mol_slice, nchunk)
    res = run_bass_kernel_spmd(nc, in_maps, list(range(NCORES)), **spmd_kwargs)
    return _postprocess(res.results, nchunk), res


def kernel(node_features, residual_features, w, mol_slice):
    out, _ = run(node_features, residual_features, w, mol_slice)
    return out


# revision 11
# speedup vs baseline: 33.0140x; 33.0140x over previous
"""Trainium2 Bass kernel for nn_BlockEnd_53266184405691.

Computes, for b in [0, 4096):
    y[b] = relu(residual[b] @ w + node[b]) row-masked so rows a >= M_b are 0
with B=4096, A=RF=F=128, fp32.

Strategy (ragged-aware): rows a >= M_b are zero by definition, so only the
valid rows (sum(M) of them, ~half on average) are processed. The host packs
valid rows into a dense stream, shards it across the 8 NeuronCores, and the
device runs a dense pipeline with no masking:
    psum = packed_residual_rows^T.T @ w    (PE, fp32)
    z    = psum + packed_node_rows         (DVE)
    out  = relu(z)                         (ACT)
The output is scattered back into a zero array on host. Packed inputs are
arranged chunk-major [chunk, 128-partition, free] so every DMA is a fully
contiguous 4MB transfer with 8KB runs per partition.
"""

import numpy as np

B, A, RF, F = 4096, 128, 128, 128
NCORES = 8
JB = 16                          # 128-row tiles per chunk
CW = JB * F                      # 2048 free-dim elements per chunk tile
ROWS_PER_CHUNK = JB * 128        # 2048 rows
XC = 2                           # chunks per DMA: 4MB transfers

_nc_cache = {}


def _build_nc(nchunk, repeat=1):
    import concourse.bacc as bacc
    import concourse.mybir as mybir
    import concourse.tile as tile

    dt = mybir.dt.float32

    # Bacc (not raw Bass): its compile() runs move_matmul_waits_to_ldweights
    # + generate_event_semaphores, which legalize multi-sem waits down to the
    # 1-wait-per-instruction TRN2 codegen limit.
    nc = bacc.Bacc("TRN2", target_bir_lowering=False, debug=False,
                   num_devices=NCORES)
    nodec = nc.dram_tensor("nodec", [nchunk, A, CW], dt, kind="ExternalInput")
    residc = nc.dram_tensor("residc", [nchunk, RF, CW], dt, kind="ExternalInput")
    w_d = nc.dram_tensor("w", [RF, F], dt, kind="ExternalInput")
    outc = nc.dram_tensor("outc", [nchunk, A, CW], dt, kind="ExternalOutput")

    with tile.TileContext(nc) as tc:
        with (
            tc.tile_pool(name="const", bufs=1) as constp,
            tc.tile_pool(name="node", bufs=3) as nodep,
            tc.tile_pool(name="resid", bufs=3) as residp,
            tc.tile_pool(name="out", bufs=3) as outp,
            tc.tile_pool(name="z", bufs=6) as zp,
            tc.tile_pool(name="psum", bufs=6, space="PSUM") as psump,
        ):
            w_sb = constp.tile([RF, F], dt)
            nc.sync.dma_start(w_sb[:], w_d[:])

            def chunk_compute(c, i, n_t, r_t, o_t):
                for g in range(JB // 4):
                    ps = psump.tile([A, 4 * F], dt)  # one PSUM bank: 4 tiles
                    for u in range(4):
                        j = g * 4 + u
                        nc.tensor.matmul(
                            ps[:, u * F:(u + 1) * F],
                            r_t[:, i, j * A:(j + 1) * A],
                            w_sb[:],
                            start=True, stop=True,
                        )
                    z = zp.tile([A, 4 * F], dt)
                    nc.vector.tensor_add(
                        z[:], ps[:], n_t[:, i, g * 4 * F:(g + 1) * 4 * F])
                    nc.scalar.activation(
                        o_t[:, i, g * 4 * F:(g + 1) * 4 * F],
                        z[:],
                        mybir.ActivationFunctionType.Relu,
                    )

            def body():
                cb = 0
                while cb < nchunk:
                    xc = min(XC, nchunk - cb)
                    n_t = nodep.tile([A, XC, CW], dt, tag="n")
                    nc.sync.dma_start(
                        n_t[:, :xc, :],
                        nodec[cb:cb + xc].rearrange("i p x -> p i x"))
                    r_t = residp.tile([RF, XC, CW], dt, tag="r")
                    nc.sync.dma_start(
                        r_t[:, :xc, :],
                        residc[cb:cb + xc].rearrange("i p x -> p i x"))
                    o_t = outp.tile([A, XC, CW], dt, tag="o")
                    for i in range(xc):
                        chunk_compute(cb + i, i, n_t, r_t, o_t)
                    nc.scalar.dma_start(
                        outc[cb:cb + xc].rearrange("i p x -> p i x"),
                        o_t[:, :xc, :])
                    cb += xc

            if repeat == 1:
                body()
            else:
                # On-device timing loop: output is overwritten identically
                # each iteration, so the kernel stays correct.
                with tc.For_i(0, repeat, 1):
                    body()
    nc.finalize()
    return nc


def _get_nc(nchunk, repeat=1):
    key = (nchunk, repeat)
    if key not in _nc_cache:
        _nc_cache[key] = _build_nc(nchunk, repeat)
    return _nc_cache[key]


def _prep_inputs(node_features, residual_features, w, mol_slice):
    """Pack valid rows, shard across cores, rearrange chunk-major.

    Returns (in_maps, meta) where meta = (idx, n_valid, nchunk, total_shape).
    """
    node_features = np.ascontiguousarray(node_features, dtype=np.float32)
    residual_features = np.ascontiguousarray(residual_features, dtype=np.float32)
    w = np.ascontiguousarray(w, dtype=np.float32)
    b, a, f = node_features.shape
    M = np.clip(np.asarray(mol_slice)[:, 0].astype(np.int64), 0, a)

    # flat indices of valid rows: (batch, atom<M_b)
    idx = np.repeat(np.arange(b, dtype=np.int64) * a, M)
    offs = np.concatenate([np.arange(m, dtype=np.int64) for m in M]) \
        if b else np.zeros(0, np.int64)
    idx = idx + offs
    n_valid = idx.shape[0]

    rows_per_core_unit = ROWS_PER_CHUNK * NCORES
    nchunk = max(1, -(-n_valid // rows_per_core_unit))
    p_total = nchunk * rows_per_core_unit

    rows_n = np.zeros((p_total, f), dtype=np.float32)
    rows_n[:n_valid] = node_features.reshape(b * a, f)[idx]
    rows_r = np.zeros((p_total, residual_features.shape[2]), dtype=np.float32)
    rows_r[:n_valid] = residual_features.reshape(b * a, -1)[idx]

    # nodec[i, c, k, j*F+x] = rows_n[(((i*nchunk)+c)*JB + j)*128 + k, x]
    nodec = np.ascontiguousarray(
        rows_n.reshape(NCORES, nchunk, JB, 128, f)
        .transpose(0, 1, 3, 2, 4)
        .reshape(NCORES, nchunk, 128, JB * f)
    )
    # residc[i, c, r, j*128+k] = rows_r[...row..., r]  (transposed per tile)
    residc = np.ascontiguousarray(
        rows_r.reshape(NCORES, nchunk, JB, 128, -1)
        .transpose(0, 1, 4, 2, 3)
        .reshape(NCORES, nchunk, -1, JB * 128)
    )
    in_maps = [
        {"nodec": nodec[i], "residc": residc[i], "w": w}
        for i in range(NCORES)
    ]
    meta = (idx, n_valid, nchunk, (b, a, f))
    return in_maps, meta


def _postprocess(results, meta):
    idx, n_valid, nchunk, (b, a, f) = meta
    rows = np.concatenate([
        np.asarray(r["outc"], dtype=np.float32)
        .reshape(nchunk, a, JB, f).transpose(0, 2, 1, 3).reshape(-1, f)
        for r in results
    ], axis=0)
    out = np.zeros((b * a, f), dtype=np.float32)
    out[idx] = rows[:n_valid]
    return out.reshape(b, a, f)


def run(node_features, residual_features, w, mol_slice, repeat=1,
        **spmd_kwargs):
    from concourse.bass_utils import run_bass_kernel_spmd

    in_maps, meta = _prep_inputs(node_features, residual_features, w, mol_slice)
    nc = _get_nc(meta[2], repeat)
    res = run_bass_kernel_spmd(nc, in_maps, list(range(NCORES)), **spmd_kwargs)
    return _postprocess(res.results, meta), res, meta


def kernel(node_features, residual_features, w, mol_slice):
    out, _, _ = run(node_features, residual_features, w, mol_slice)
    return out
